# revision 20
# baseline (speedup 1.0000x reference)
"""Trainium2 Bass kernel for nn_BiGNN_53772990546511 (v3).

Restructure over v2 (85.8us baseline):
  - Outputs leave the device transposed/compressed: day-0/1 user rows as
    [feat, user] blocks, day>=2 user rows + all loc rows as per-day
    column vectors in d_rows [P, 16]; the host expands at gather time.
    Output DMA drops 3.93MB -> 0.55MB; all day0u/day1u/phase-7 PE
    transposes and copies disappear.
  - phase 2 (day-0 embeddings) runs only for the LOCAL 512 users; the
    remote half is needed only through f1 (attention logits), computed
    as xw1^T @ (A*recip)  with the 1/cnt folded on the host into an f8
    copy of A (4k cycles instead of 8k).
  - f1-local = (W a1)^T @ xuavgT (2 matmuls over feats instead of 16
    over locs).
  - Inputs spread over the three DMA queues (sync/scalar/gpsimd,
    ~130GB/s each) with per-half splits so phase 2 starts earlier.
  - All [1,N] row ops moved to the scalar engine (DVE single-partition
    ops run ~6.5ns/elem; scalar ~0.8ns/elem).  The Z normalization is
    computed as CZ/Z = exp(-ln(Z/CZ)) with two scalar activations per
    half (ln/exp/copy share one activation table set); the 1/CZ factor
    is folded into the psz ones-vector.
  - The ptm mask-multiply is split between DVE and the otherwise-idle
    GpSimd engine so the attention loop stays tensor-bound.
Sharding: 8 cores = 4 batch pairs; odd cores get user-axis rotated (by
512) host tensors so local users 0..511 are global 512..1023.
"""
import numpy as np

N_USER = 1024
N_LOC = 1024
DM = 256
HD = 256
B = 4
D = 5
E = 4096
ALPHA = 0.2
CZ = 0.25          # Z scale: wh stored *1/CZ, recipZ stored CZ/Z
FB = -5.0          # per-side exp bias
P = 128
NCORES = 8
WU = 1024          # attention user width (full; both halves on device)

_CACHE = {}


# --------------------------------------------------------------------------
# Workarounds for this walrus build's 1-sync-wait-per-instruction limit.
# --------------------------------------------------------------------------
def _apply_tile_patch():
    import concourse.tile as tile
    from concourse.tile_sem_assignment import tick_to_sem

    if not getattr(tile.TileContext, "_drain_patched", False):
        def _patched(self, tick_clock, wait_clock):
            nc = self.nc
            gc = tick_clock.global_clock
            for proc, sem in self.sems.allocated().items():
                t = gc[proc]
                if t and t > 0:
                    nc.sync.nop().wait_op(sem, tick_to_sem(t, proc), "sem-ge")
            nc.sync.drain()
            nc.all_engine_barrier()
            popped = nc._tile_sem_poison_stack.pop()
            assert popped is self._sem_poison
            nc.clear_and_free_semaphores(list(self.sems.allocated().values()))
            nc.all_engine_barrier()

        tile.TileContext._drain_and_barrier = _patched
        tile.TileContext._drain_patched = True

    import json as _json
    import concourse.bass_utils as _bu
    import concourse.bass2jax as _b2j

    if not getattr(_bu, "_wait_split_patched", False):
        _orig_compile = _bu.compile_bir_kernel

        def _split_waits(bir_json):
            j = _json.loads(bir_json)
            nid = [0]
            for fn in j.get("functions", []):
                for bb in fn.get("blocks", []):
                    out = []
                    for inst in bb.get("instructions", []):
                        si = inst.get("sync_info") or {}
                        ow = si.get("on_wait") or []
                        if len(ow) > 1:
                            for w in ow[:-1]:
                                nid[0] += 1
                                out.append({
                                    "debug": inst.get("debug", 0),
                                    "engine": inst.get("engine", "SP"),
                                    "ins": [],
                                    "name": f"WSPL-{nid[0]}",
                                    "opcode": "NoOp",
                                    "outs": [],
                                    "sync_info": {"on_update": [],
                                                  "on_wait": [w]},
                                })
                            si["on_wait"] = [ow[-1]]
                        out.append(inst)
                    bb["instructions"] = out
            return _json.dumps(j).encode()

        def _patched_compile(bir_json, tmpdir, neff_name="file.neff"):
            return _orig_compile(_split_waits(bir_json), tmpdir,
                                 neff_name=neff_name)

        _bu.compile_bir_kernel = _patched_compile
        _b2j.compile_bir_kernel = _patched_compile
        _bu._wait_split_patched = True


def _build_nc():
    import contextlib
    import concourse.bass as bass
    import concourse.tile as tile
    from concourse import mybir

    _apply_tile_patch()
    f32 = mybir.dt.float32
    f16 = mybir.dt.float16
    f8 = mybir.dt.float8e4
    AF = mybir.ActivationFunctionType
    OP = mybir.AluOpType

    nc = bass.Bass(num_devices=NCORES)

    # ---------------- DRAM tensors ----------------
    d_A8T = nc.dram_tensor("A8T", [N_LOC, 512], f8, kind="ExternalInput")
    d_A8R = nc.dram_tensor("A8R", [N_LOC, 512], f8, kind="ExternalInput")
    d_MT8 = nc.dram_tensor("MT8", [N_LOC, WU], f16, kind="ExternalInput")
    d_xloc16 = nc.dram_tensor("xloc16", [N_LOC, DM], f16, kind="ExternalInput")
    d_xlocT16 = nc.dram_tensor("xlocT16", [DM, N_LOC], f16,
                               kind="ExternalInput")
    d_W16 = nc.dram_tensor("W16", [DM, HD], f16, kind="ExternalInput")
    d_WT16 = nc.dram_tensor("WT16", [HD, DM], f16, kind="ExternalInput")
    d_acolP = nc.dram_tensor("acolP", [P, 4], f16, kind="ExternalInput")
    d_gtri = nc.dram_tensor("gtri", [N_LOC, 3], f16, kind="ExternalInput")
    d_recip = nc.dram_tensor("recip", [1, 512], f16, kind="ExternalInput")
    d_nothas = nc.dram_tensor("nothas", [1, WU], f16, kind="ExternalInput")
    d_colb = nc.dram_tensor("colb", [P, 8], f32, kind="ExternalInput")
    d_outT = nc.dram_tensor("outT", [2, 2 * P, 512], f16,
                            kind="ExternalOutput")
    d_rows = nc.dram_tensor("rows", [P, 16], f16, kind="ExternalOutput")

    with tile.TileContext(nc) as tc:
        with contextlib.ExitStack() as ctx:
            persist = ctx.enter_context(tc.tile_pool(name="persist", bufs=1))
            work = ctx.enter_context(tc.tile_pool(name="work", bufs=1))
            psAcc = ctx.enter_context(
                tc.tile_pool(name="psAcc", bufs=1, space="PSUM"))
            psW = ctx.enter_context(
                tc.tile_pool(name="psW", bufs=1, space="PSUM"))
            psF = ctx.enter_context(
                tc.tile_pool(name="psF", bufs=1, space="PSUM"))
            psZ = ctx.enter_context(
                tc.tile_pool(name="psZ", bufs=1, space="PSUM"))

            def tile_dma(eng, dst, dram_t, r0, r1, c0=0, c1=None):
                """rows r0:r1 of dram_t (row-tiled by P) -> dst col range."""
                c1 = dram_t.shape[1] if c1 is None else c1
                w = c1 - c0
                t = (r1 - r0) // P
                src = dram_t[r0:r1, c0:c1].rearrange("(t p) u -> p t u", p=P)
                base = (r0 // P) * w
                eng.dma_start(
                    out=dst[:, base:base + t * w].rearrange(
                        "p (t u) -> p t u", t=t),
                    in_=src)

            # ------------- input loads (three DMA queues) -------------
            # sync: phW + phase2 criticals, finely chunked for early start
            acolP = persist.tile([P, 4], f16, name="acolP")
            nc.sync.dma_start(out=acolP[:], in_=d_acolP[:])
            WT16 = persist.tile([P, 2 * DM], f16, name="WT16")
            tile_dma(nc.sync, WT16, d_WT16, 0, 256)
            # tiny phase-2-critical tensors ride the fast sync queue
            gtri = persist.tile([P, 8 * 3], f16, name="gtri")
            tile_dma(nc.sync, gtri, d_gtri, 0, 1024)
            reciprow = persist.tile([1, 512], f16, name="reciprow")
            nc.sync.dma_start(out=reciprow[:], in_=d_recip[:])
            nothasrow = persist.tile([1, WU], f16, name="nothasrow")
            nc.sync.dma_start(out=nothasrow[:], in_=d_nothas[:])
            colb = persist.tile([P, 8], f32, name="colb")
            nc.sync.dma_start(out=colb[:], in_=d_colb[:])
            xloc16 = persist.tile([P, 8 * DM], f16, name="xloc16")
            A8 = persist.tile([P, 8 * 512], f8, name="A8")
            for r0, r1 in ((0, 256), (256, 512), (512, 1024)):
                tile_dma(nc.sync, xloc16, d_xloc16, r0, r1)
                tile_dma(nc.sync, A8, d_A8T, r0, r1)
            # scalar: Wext + A8R + xlocT16 (needed mid-kernel)
            Wext = persist.tile([P, 2 * 258], f16, name="Wext")
            for kt in range(2):
                nc.scalar.dma_start(
                    out=Wext[:, kt * 258:kt * 258 + 256],
                    in_=d_W16[kt * P:(kt + 1) * P, :])
            xlocT16 = persist.tile([P, 2 * N_LOC], f16, name="xlocT16")
            tile_dma(nc.scalar, xlocT16, d_xlocT16, 0, 256)
            A8R = persist.tile([P, 8 * 512], f8, name="A8R")
            tile_dma(nc.scalar, A8R, d_A8R, 0, 1024)
            # MT (needed only from phase 5/6): first half on sync after A8,
            # second half on gpsimd (that queue runs ~40GB/s - slack only)
            MT = persist.tile([P, 8 * WU], f16, name="MT")
            tile_dma(nc.sync, MT, d_MT8, 0, 512)
            tile_dma(nc.gpsimd, MT, d_MT8, 512, 1024)

            # ------------- constants -------------
            # psz lhsT carries 1/CZ so zz accumulates Z/CZ directly
            ones16 = persist.tile([P, 1], f16, name="ones16")
            nc.vector.memset(ones16[:], 1.0 / CZ)
            one11 = persist.tile([1, 1], f16, name="one11")
            nc.vector.memset(one11[:], 1.0 / CZ)
            onesrow16 = persist.tile([1, P], f16, name="onesrow16")
            nc.vector.memset(onesrow16[:], 1.0)
            onesrow32 = persist.tile([1, P], f32, name="onesrow32")
            nc.vector.memset(onesrow32[:], 1.0)
            fbcol = persist.tile([P, 1], f32, name="fbcol")
            nc.vector.memset(fbcol[:], FB)

            def Wk(kt, mh):
                return Wext[:, kt * 258 + mh * P:kt * 258 + (mh + 1) * P]

            # ---------------- phase W: wa pairs + Wext ----------------
            wapair = [persist.tile([P, 2], f16, name=f"wap{i}")
                      for i in range(2)]
            for mh in range(2):
                ps = psF.tile([P, 512], f32, name="pswa", tag="f")
                ps = ps[:, 2 * mh:2 * mh + 2]
                for kt in range(2):
                    nc.tensor.matmul(
                        ps[:],
                        WT16[:, kt * DM + mh * P:kt * DM + (mh + 1) * P],
                        acolP[:, 2 * kt:2 * kt + 2],
                        start=(kt == 0), stop=(kt == 1))
                nc.vector.tensor_copy(wapair[mh][:], ps[:])
            for kt in range(2):
                nc.vector.tensor_copy(Wext[:, kt * 258 + 256:kt * 258 + 258],
                                      wapair[kt][:])

            # ---- local broadcasts of recip / nothas rows ----
            recipB = persist.tile([P, 512], f16, name="recipB")
            nothasB = persist.tile([P, 512], f16, name="nothasB")
            for dst, row in ((recipB, reciprow[0:1, :]),
                             (nothasB, nothasrow[0:1, 0:512])):
                ps = psF.tile([P, 512], f32, name="bcr", tag="f")
                nc.tensor.matmul(ps[:], onesrow16[:], row,
                                 start=True, stop=True)
                nc.scalar.activation(dst[:], ps[:], AF.Copy)

            # ---------------- phase 2 (local): xu sums + g-sums ---------
            psN = [psAcc.tile([P, WU], f32, name=f"nf{fh}", tag=f"nf{fh}")
                   for fh in range(2)]
            psG = [psZ.tile([P, 512], f32, name="psG0", tag="z"),
                   psF.tile([P, 512], f32, name="psG1", tag="f")]
            for lt in range(8):
                for fh in range(2):
                    nc.tensor.matmul(
                        psN[fh][:, 0:512],
                        xloc16[:, lt * DM + fh * P:lt * DM + (fh + 1) * P],
                        A8[:, lt * 512:(lt + 1) * 512],
                        start=(lt == 0), stop=(lt == 7),
                        skip_group_check=True)
                    nc.tensor.matmul(
                        psG[fh][:, 0:3],
                        xloc16[:, lt * DM + fh * P:lt * DM + (fh + 1) * P],
                        gtri[:, lt * 3:(lt + 1) * 3],
                        start=(lt == 0), stop=(lt == 7),
                        skip_group_check=True)
            # xuavgT = raw sums * recip; day0 fixup in transposed space
            xuavgT = persist.tile([P, 2 * 512], f16, name="xuavgT")
            mwe16 = [persist.tile([P, 1], f16, name=f"mwe16{fh}")
                     for fh in range(2)]
            mcol16 = [persist.tile([P, 1], f16, name=f"mcol16{fh}")
                      for fh in range(2)]
            day0T = work.tile([P, 2 * 512], f16, name="day0T", tag="day0T")
            for fh in range(2):
                nc.vector.tensor_copy(mwe16[fh][:], psG[fh][:, 0:1])
                t1 = work.tile([P, 1], f32, name=f"t1{fh}", tag=f"t1{fh}")
                nc.vector.tensor_scalar(
                    out=t1[:], in0=psG[fh][:, 2:3], scalar1=2.0 / 3072.0,
                    scalar2=0.0, op0=OP.mult, op1=OP.add)
                nc.vector.scalar_tensor_tensor(
                    out=t1[:], in0=psG[fh][:, 1:2], scalar=1.0 / 3072.0,
                    in1=t1[:], op0=OP.mult, op1=OP.add)
                nc.vector.scalar_tensor_tensor(
                    out=mcol16[fh][:], in0=psG[fh][:, 0:1],
                    scalar=colb[:, 6:7], in1=t1[:], op0=OP.mult, op1=OP.add)
                sl = slice(fh * 512, (fh + 1) * 512)
                nc.vector.tensor_tensor(
                    out=xuavgT[:, sl], in0=psN[fh][:, 0:512],
                    in1=recipB[:], op=OP.mult)
                nc.vector.scalar_tensor_tensor(
                    out=day0T[:, sl], in0=nothasB[:],
                    scalar=mwe16[fh][:], in1=xuavgT[:, sl],
                    op0=OP.mult, op1=OP.add)
            nc.scalar.dma_start(
                out=d_outT[0].rearrange("(t p) u -> p t u", p=P),
                in_=day0T[:].rearrange("p (t u) -> p t u", t=2))

            # ---------------- phase 1: xw pairs ----------------
            psf = psZ.tile([P, 512], f32, name="psf", tag="z")
            f1row16 = persist.tile([1, WU], f16, name="f1row16")
            psxw = psW.tile([P, 512], f32, name="psxw", tag="a0")
            for lt in range(8):
                ps = psxw[:, 2 * lt:2 * lt + 2]
                for kt in range(2):
                    nc.tensor.matmul(
                        ps,
                        xlocT16[:, kt * N_LOC + lt * P:
                                kt * N_LOC + (lt + 1) * P],
                        wapair[kt][:],
                        start=(kt == 0), stop=(kt == 1),
                        skip_group_check=True)
            xw1c = persist.tile([P, 8], f16, name="xw1c")
            nc.vector.tensor_copy(xw1c[:, 0:8], psxw[:, 0:16:2])
            xw2c = persist.tile([P, 8], f32, name="xw2c")
            nc.vector.tensor_copy(xw2c[:, 0:8], psxw[:, 1:16:2])
            u1col = persist.tile([P, 8], f32, name="u1col")
            u2col = persist.tile([P, 8], f32, name="u2col")
            nc.scalar.activation(u1col[:], xw2c[:], AF.Exp, bias=fbcol[:],
                                 scale=1.0)
            nc.scalar.activation(u2col[:], xw2c[:], AF.Exp, bias=fbcol[:],
                                 scale=ALPHA)

            # ---------------- f1 remote (via recip-scaled A) ------------
            for lt in range(8):
                nc.tensor.matmul(
                    psf[32:33, 0:512], xw1c[:, lt:lt + 1],
                    A8R[:, lt * 512:(lt + 1) * 512],
                    start=(lt == 0), stop=(lt == 7),
                    skip_group_check=True)
            nc.scalar.activation(f1row16[0:1, 512:1024], psf[32:33, 0:512],
                                 AF.Copy)

            # ---------------- f1 local (via xuavgT) ----------------
            for fh in range(2):
                nc.tensor.matmul(
                    psf[0:1, 0:512], wapair[fh][:, 0:1],
                    xuavgT[:, fh * 512:(fh + 1) * 512],
                    start=(fh == 0), stop=(fh == 1),
                    skip_group_check=True)
            nc.scalar.activation(f1row16[0:1, 0:512], psf[0:1, 0:512],
                                 AF.Copy)

            # ---------------- V1B/V2B broadcast + exp ----------------
            V1B = persist.tile([P, WU], f16, name="V1B")
            V2B = persist.tile([P, WU], f16, name="V2B")
            for uc in (1, 0):
                ps = psF.tile([P, 512], f32, name="psVB", tag="f")
                nc.tensor.matmul(ps[:], onesrow16[:],
                                 f1row16[0:1, uc * 512:(uc + 1) * 512],
                                 start=True, stop=True)
                usl = slice(uc * 512, (uc + 1) * 512)
                nc.scalar.activation(V1B[:, usl], ps[:], AF.Exp,
                                     bias=fbcol[:], scale=1.0)
                nc.scalar.activation(V2B[:, usl], ps[:], AF.Exp,
                                     bias=fbcol[:], scale=ALPHA)

            # ---------------- whext: Wh per loc (feeds psN) -------------
            wh16 = persist.tile([P, 8 * HD], f16, name="wh16")
            for lt in range(8):
                ps = psW.tile([P, 512], f32, name="whx", tag=f"a{lt % 2}")
                ps = ps[:, 0:256]
                for kt in range(2):
                    nc.tensor.matmul(
                        ps,
                        xlocT16[:, kt * N_LOC + lt * P:
                                kt * N_LOC + (lt + 1) * P],
                        Wext[:, kt * 258:kt * 258 + 256],
                        start=(kt == 0), stop=(kt == 1))
                nc.scalar.activation(wh16[:, lt * HD:(lt + 1) * HD], ps,
                                     AF.Copy, scale=1.0 / CZ)

            # ---------------- mw0 row + v1 columns ----------------
            psmr = psF.tile([P, 512], f32, name="psmr", tag="f")
            pr = psmr[0:1, 0:HD]
            for kt in range(2):
                nc.tensor.matmul(pr, mcol16[kt][:],
                                 Wext[:, kt * 258:kt * 258 + 256],
                                 start=(kt == 0), stop=(kt == 1),
                                 skip_group_check=True)
            mw0row16 = persist.tile([1, HD], f16, name="mw0row16")
            nc.scalar.activation(mw0row16[:], pr, AF.Copy, scale=1.0 / CZ)
            rows_out = persist.tile([P, 16], f16, name="rows_out")
            v1col16 = [rows_out[:, mh:mh + 1] for mh in range(2)]
            psmw = psF.tile([P, 512], f32, name="psmw", tag="f")
            for mh in range(2):
                ps = psmw[:, mh:mh + 1]
                for kt in range(2):
                    nc.tensor.matmul(ps, Wk(kt, mh), mcol16[kt][:],
                                     start=(kt == 0), stop=(kt == 1),
                                     skip_group_check=True)
                nc.vector.tensor_scalar(out=v1col16[mh], in0=ps,
                                        scalar1=1.0, scalar2=0.0,
                                        op0=OP.mult, op1=OP.max)

            # ---------------- phase 5+6: attention ----------------
            zz = psZ.tile([P, 512], f32, name="zz", tag="z")
            psz = [zz[32 * uc:32 * uc + 1, 0:512] for uc in range(2)]
            for lt in range(8):
                x1m = work.tile([P, WU], f16, name="x1m", tag="x1m",
                                bufs=2)
                nc.scalar.activation(x1m[:], V1B[:], AF.Copy,
                                     scale=u1col[:, lt:lt + 1])
                x2x = work.tile([P, WU], f16, name="x2x", tag="x2x",
                                bufs=2)
                nc.vector.scalar_tensor_tensor(
                    out=x2x[:], in0=V2B[:], scalar=u2col[:, lt:lt + 1],
                    in1=x1m[:], op0=OP.mult, op1=OP.max)
                # ptm alternates engines at full width: partial-width DVE
                # slices lose the fast path (384 cols cost MORE than 512)
                ptm = work.tile([P, WU], f16, name="ptm", tag="ptm",
                                bufs=3)
                eng = nc.vector if lt % 2 == 0 else nc.gpsimd
                eng.tensor_tensor(out=ptm[:], in0=x2x[:],
                                  in1=MT[:, lt * WU:(lt + 1) * WU],
                                  op=OP.mult)
                for fh in range(2):
                    for uc in range(2):
                        nc.tensor.matmul(
                            psN[fh][:, uc * 512:(uc + 1) * 512],
                            wh16[:, lt * HD + fh * P:lt * HD + (fh + 1) * P],
                            ptm[:, uc * 512:(uc + 1) * 512],
                            start=(lt == 0), stop=False,
                            skip_group_check=True)
                for uc in range(2):
                    nc.tensor.matmul(psz[uc], ones16[:],
                                     ptm[:, uc * 512:(uc + 1) * 512],
                                     start=(lt == 0), stop=False)
            # fixups: psz first so the Z chain can start while psN finishes
            for uc in range(2):
                nc.tensor.matmul(psz[uc], one11[:],
                                 nothasrow[0:1, uc * 512:(uc + 1) * 512],
                                 start=False, stop=True)
            for fh in range(2):
                for uc in range(2):
                    nc.tensor.matmul(
                        psN[fh][:, uc * 512:(uc + 1) * 512],
                        mw0row16[0:1, fh * P:(fh + 1) * P],
                        nothasrow[0:1, uc * 512:(uc + 1) * 512],
                        start=False, stop=True,
                        skip_group_check=True)

            # CZ/Z = exp(-ln(Z/CZ)): ln on the psz rows (scalar handles
            # [1,N] at ~0.8ns/elem), broadcast the ln row, exp fused into
            # the broadcast copy.  ln/exp/copy share one act table set.
            lnrow32 = persist.tile([1, WU], f32, name="lnrow32")
            rzB = persist.tile([P, WU], f16, name="rzB")
            for uc in range(2):
                nc.scalar.activation(lnrow32[0:1, uc * 512:(uc + 1) * 512],
                                     psz[uc], AF.Ln)
                ps = psF.tile([P, 512], f32, name="psZB", tag="f")
                nc.tensor.matmul(ps[:], onesrow32[:],
                                 lnrow32[0:1, uc * 512:(uc + 1) * 512],
                                 start=True, stop=True)
                nc.scalar.activation(rzB[:, uc * 512:(uc + 1) * 512],
                                     ps[:], AF.Exp, scale=-1.0)
            # h1uT = relu(num) * CZ/Z in quarters so the day-1 output DMA
            # (local half = uc 0) leaves as soon as rzB-uc0 is ready
            h1uT = persist.tile([P, 2 * WU], f16, name="h1uT")
            scur = persist.tile([P, 2], f32, name="scur")
            scq = [[work.tile([P, 1], f32, name=f"scq{fh}{uc}",
                              tag=f"scq{fh}{uc}") for uc in range(2)]
                   for fh in range(2)]
            for uc in range(2):
                for fh in range(2):
                    nc.vector.scalar_tensor_tensor(
                        out=h1uT[:, fh * WU + uc * 512:
                                 fh * WU + (uc + 1) * 512],
                        in0=psN[fh][:, uc * 512:(uc + 1) * 512], scalar=0.0,
                        in1=rzB[:, uc * 512:(uc + 1) * 512],
                        op0=OP.max, op1=OP.mult,
                        accum_out=scq[fh][uc][:])
                    if uc == 0:
                        nc.scalar.dma_start(
                            out=d_outT[1, fh * P:(fh + 1) * P, :],
                            in_=h1uT[:, fh * WU:fh * WU + 512])
            for fh in range(2):
                nc.vector.tensor_tensor(out=scur[:, fh:fh + 1],
                                        in0=scq[fh][0][:], in1=scq[fh][1][:],
                                        op=OP.add)

            # ---------------- phase 7: days 2..4 ----------------
            vcol = v1col16
            scol = [scur[:, fh:fh + 1] for fh in range(2)]
            for day in (2, 3, 4):
                dd = day - 2
                base = 2 + dd * 4
                pair = [work.tile([P, 2], f16, name=f"pair{day}{fh}",
                                  tag=f"pair{fh}", bufs=2)
                        for fh in range(2)]
                for fh in range(2):
                    nc.vector.tensor_copy(pair[fh][:, 0:1], vcol[fh])
                    t2 = work.tile([P, 1], f32, name=f"t2{day}{fh}",
                                   tag=f"t2{fh}", bufs=2)
                    nc.vector.tensor_scalar(out=t2[:], in0=scol[fh],
                                            scalar1=1.0 / 3072.0, scalar2=0.0,
                                            op0=OP.mult, op1=OP.add)
                    nc.vector.scalar_tensor_tensor(
                        out=pair[fh][:, 1:2], in0=vcol[fh],
                        scalar=2048.0 / 3072.0, in1=t2[:],
                        op0=OP.mult, op1=OP.add)
                rv = [rows_out[:, base + 2 * mh:base + 2 * mh + 2]
                      for mh in range(2)]
                pswp = psF.tile([P, 512], f32, name="pswp", tag="f")
                for mh in range(2):
                    ps = pswp[:, 2 * mh:2 * mh + 2]
                    for kt in range(2):
                        nc.tensor.matmul(ps, Wk(kt, mh), pair[kt][:],
                                         start=(kt == 0), stop=(kt == 1),
                                         skip_group_check=True)
                    nc.vector.tensor_scalar(out=rv[mh], in0=ps,
                                            scalar1=1.0, scalar2=0.0,
                                            op0=OP.mult, op1=OP.max)
                if day < 4:
                    vcol = [rv[mh][:, 1:2] for mh in range(2)]
                    nscur = [work.tile([P, 1], f32, name=f"ns{day}{fh}",
                                       tag=f"ns{fh}", bufs=2)
                             for fh in range(2)]
                    for fh in range(2):
                        t3 = work.tile([P, 1], f32, name=f"t3{day}{fh}",
                                       tag=f"t3{fh}", bufs=2)
                        nc.vector.tensor_scalar(
                            out=t3[:], in0=rv[fh][:, 1:2],
                            scalar1=colb[:, dd * 2 + 1:dd * 2 + 2],
                            scalar2=0.0, op0=OP.mult, op1=OP.add)
                        nc.vector.scalar_tensor_tensor(
                            out=nscur[fh][:], in0=rv[fh][:, 0:1],
                            scalar=colb[:, dd * 2:dd * 2 + 1], in1=t3[:],
                            op0=OP.mult, op1=OP.add)
                    scol = [nscur[fh][:] for fh in range(2)]
            nc.gpsimd.dma_start(out=d_rows[:], in_=rows_out[:])

    return nc


def _host_prep(x_loc, mob_links, text_links, W, a):
    """Index-only preprocessing -> per-core input maps."""
    import ml_dtypes
    f8 = ml_dtypes.float8_e4m3

    x_loc = np.ascontiguousarray(x_loc, np.float32)
    W = np.ascontiguousarray(W, np.float32)
    a = np.ascontiguousarray(a, np.float32).reshape(2 * HD)
    mob = np.asarray(mob_links)
    text = np.asarray(text_links)

    a1, a2 = a[:HD], a[HD:]
    acolP = np.stack([a1[:P], a2[:P], a1[P:], a2[P:]], axis=1)
    shared = {
        "xloc16": x_loc.astype(np.float16),
        "xlocT16": np.ascontiguousarray(x_loc.T).astype(np.float16),
        "W16": W.astype(np.float16),
        "WT16": np.ascontiguousarray(W.T).astype(np.float16),
        "acolP": np.ascontiguousarray(acolP).astype(np.float16),
    }

    in_maps = []
    masks = []
    for c in range(NCORES):
        b, r = c // 2, c % 2
        rot = r * 512
        u0 = np.concatenate([mob[b, 0, :, 0], text[b, 0, :, 0]]).astype(
            np.int64)
        l0 = np.concatenate([mob[b, 0, :, 1], text[b, 0, :, 1]]).astype(
            np.int64)
        cnt = np.bincount(u0, minlength=N_USER).astype(np.float32)
        A = np.zeros((N_USER, N_LOC), np.float32)
        np.add.at(A, (u0, l0), 1.0)
        Mb = np.zeros((N_USER, N_LOC), np.float32)
        Tb = np.zeros((N_USER, N_LOC), np.float32)
        Mb[mob[b, 0, :, 0], mob[b, 0, :, 1]] = 1.0
        Tb[text[b, 0, :, 0], text[b, 0, :, 1]] = 1.0
        M = Mb + Tb
        recip = 1.0 / np.maximum(cnt, 1.0)
        has0 = (cnt > 0).astype(np.float32)
        n_with = max(float(has0.sum()), 1.0)
        nh_cnt = float(N_USER) - float(has0.sum())

        def rollu(x, axis=0):
            return np.roll(x, -rot, axis=axis)

        colb = np.zeros((P, 8), np.float32)
        hE_days = []
        for dd in range(3):
            us = np.concatenate([mob[b, dd + 1, :, 0], text[b, dd + 1, :, 0]])
            hE = np.zeros(N_USER, np.float32)
            hE[us] = 1.0
            hE_days.append(hE)
            colb[:, dd * 2] = hE.sum()
            colb[:, dd * 2 + 1] = N_USER - hE.sum()
        colb[:, 6] = nh_cnt / 3072.0

        Ar = rollu(A, 0)
        ArT = np.ascontiguousarray(Ar.T)
        AR = np.ascontiguousarray((Ar * rollu(recip)[:, None]).T[:, 512:])
        m = dict(shared)
        m.update({
            "A8T": ArT[:, :512].astype(f8),
            "A8R": AR.astype(f8),
            "MT8": np.ascontiguousarray(rollu(M, 0).T).astype(np.float16),
            "recip": rollu(recip)[None, :512].astype(np.float16),
            "nothas": rollu(1.0 - has0)[None, :].astype(np.float16),
            "gtri": np.stack([
                (has0 * recip / n_with) @ A,
                recip @ A,
                np.ones(N_LOC, np.float32)], axis=1).astype(np.float16),
            "colb": colb,
        })
        in_maps.append(m)
        masks.append(hE_days)
    return in_maps, masks


def kernel(**inputs):
    from concourse.bass_utils import run_bass_kernel_spmd

    if "nc" not in _CACHE:
        _CACHE["nc"] = _build_nc()
    nc = _CACHE["nc"]

    x_loc = np.ascontiguousarray(inputs["x_loc"], np.float32)
    in_maps, masks = _host_prep(x_loc, inputs["mob_links"],
                                inputs["text_links"], inputs["W"],
                                inputs["a"])
    res = run_bass_kernel_spmd(nc, in_maps, core_ids=list(range(NCORES)))

    out = np.zeros((B, D, N_USER + 2 * N_LOC, HD), np.float32)
    for c in range(NCORES):
        b, r = c // 2, c % 2
        oT = np.asarray(res.results[c]["outT"], np.float32)
        R = np.asarray(res.results[c]["rows"], np.float32)
        us = slice(r * 512, (r + 1) * 512)
        out[b, 0, us, :] = oT[0].T
        out[b, 1, us, :] = oT[1].T
        if r == 0:
            out[b, 0, N_USER:, :] = np.tile(x_loc, (2, 1))
            v1 = np.concatenate([R[:, 0], R[:, 1]])
            out[b, 1, N_USER:, :] = v1[None, :]
        for day in (2, 3, 4):
            base = 2 + (day - 2) * 4
            e_row = np.concatenate([R[:, base], R[:, base + 2]])
            n_row = np.concatenate([R[:, base + 1], R[:, base + 3]])
            hE = masks[c][day - 2][r * 512:(r + 1) * 512]
            out[b, day, us, :] = np.where(hE[:, None] > 0, e_row[None, :],
                                          n_row[None, :])
            if r == 0:
                out[b, day, N_USER:, :] = n_row[None, :]
    return out
